# revision 17
# baseline (speedup 1.0000x reference)
"""Multi-head attention kernel for Trainium2, 8-core tensor/data parallel.

Problem: x[2,2048,1024] -> qkv proj (w_qkv [1024,3072]) -> 16-head attention
         -> out proj (w_proj [1024,1024]) + b_proj.

Sharding: core c handles batch b=c//4 and heads 4*(c%4)..4*(c%4)+4.
Each core computes a partial output Y^T = w_proj_rows^T @ OH (its 4 heads'
contribution, transposed); the host sums the 4 partials per batch,
transposes, and adds the bias.

Schedule: the kernel is ACT-engine bound (softmax exp is ~147us of scalar
engine time per core and exp only runs there), so everything is organized
as a single slot stream paced by the exp chain.  Each slot emits one
S^T matmul pair (row-tiled K=64 halves that run concurrently on the PE
array), the exp for that tile, and the PV matmul pair for the slot LAG
positions earlier; Q/K/V projections and the output projection are woven
into the remaining PE capacity between slots using a ns-budget model so
the PE queue never runs ahead of the ACT queue (which would stall exp).
All data is bf16 (inputs, weights, activations, output partials); PSUM
accumulation stays fp32.  Softmax skips max-subtraction (scores are
~N(0,1) after the 1/sqrt(D) scale) and folds the row-sum into the PV
matmul via an appended ones-column on V; denominators use the fast
approximate reciprocal (~18 bits, plenty for this tolerance).
"""

from contextlib import ExitStack

import numpy as np

import concourse.bass as bass
import concourse.mybir as mybir
from concourse import bacc, tile

B, N, C, H = 2, 2048, 1024, 16
D = C // H            # 64 head dim
SCALE = float(D) ** -0.5
HPC = 4               # heads per core
HD = HPC * D          # 256 head-dim columns per core
NCORES = 8

F32 = mybir.dt.float32
F32R = mybir.dt.float32r
BF16 = mybir.dt.bfloat16

QT = N // 128         # 16 query/key 128-tiles
CT = C // 128         # 8 channel 128-tiles
QB = N // 512         # 4 query 512-blocks
HDT = HD // 128       # 2 head-pair tiles (2 heads of 64 each)

NIT = HDT * QB        # 8 attention iterations (ht-major)
LAG = 8               # PV trails its exp by this many slots

# empirical effective PE costs (ns), from HW trace at nominal clock
SPAIR_NS = 390.0      # row-tiled S matmul pair (2x [64]x512)
PV_NS = 225.0         # one PV matmul, ap=512
PROJ_MM_NS = 240.0    # projection matmul, ap=512
RB_NS = 250.0         # ones-broadcast matmul
EXP_NS = 1147.0       # ACT ns per [128,2,512] exp
AHEAD_NS = 1200.0     # how far PE emission may run ahead of ACT


def _build():
    nc = bacc.Bacc(None)
    xT = nc.declare_dram_parameter("xT", [C, N], BF16, isOutput=False)
    wkq = nc.declare_dram_parameter("wkq", [C, 2 * HD], BF16,
                                    isOutput=False)
    wv = nc.declare_dram_parameter("wv", [C, HD], BF16, isOutput=False)
    wp = nc.declare_dram_parameter("wp", [HD, C], BF16, isOutput=False)
    yT = nc.declare_dram_parameter("yT", [C, N], BF16, isOutput=True)
    bdbg = nc.declare_dram_parameter("bdbg", [64, 512], F32, isOutput=True)

    with tile.TileContext(nc) as tc, ExitStack() as ctx:
        const_pool = ctx.enter_context(tc.tile_pool(name="const", bufs=1))
        w_pool = ctx.enter_context(tc.tile_pool(name="w", bufs=1))
        x_pool = ctx.enter_context(tc.tile_pool(name="x", bufs=1))
        qk_pool = ctx.enter_context(tc.tile_pool(name="qk", bufs=1))
        vo_pool = ctx.enter_context(tc.tile_pool(name="vo", bufs=1))
        oht_pool = ctx.enter_context(tc.tile_pool(name="oht", bufs=1))
        pt_pool = ctx.enter_context(tc.tile_pool(name="pt", bufs=1))
        small_pool = ctx.enter_context(tc.tile_pool(name="small", bufs=2))
        out_pool = ctx.enter_context(tc.tile_pool(name="out", bufs=2))
        st_pool = ctx.enter_context(
            tc.tile_pool(name="ps_st", bufs=2, space="PSUM"))
        ot_pool = ctx.enter_context(
            tc.tile_pool(name="ps_ot", bufs=1, space="PSUM"))
        proj_pool = ctx.enter_context(
            tc.tile_pool(name="ps_proj", bufs=2, space="PSUM"))

        ones_f = const_pool.tile([128, 64], F32)
        nc.vector.memset(ones_f, 1.0)
        ones_b = const_pool.tile([128, 64], BF16)
        nc.vector.memset(ones_b, 1.0)

        # ---- input DMAs, ordered by first use ----
        xq0_t = [x_pool.tile([128, CT // 2, 512], BF16, name=f"xq0{h}",
                             tag=f"xq0{h}") for h in range(2)]
        xq_t = [None] + [x_pool.tile([128, CT, 512], BF16, name=f"xq{qb}",
                                     tag=f"xq{qb}") for qb in range(1, QB)]
        wtiles = {}

        def xq(qb, ct):
            if qb == 0:
                return xq0_t[ct // (CT // 2)][:, ct % (CT // 2), :]
            return xq_t[qb][:, ct, :]

        # split input DMAs across the two hw DMA-gen engines (sync + scalar)
        # so transfers run on two queues in parallel; ACT is idle at startup
        def dma_w(name, dram, eng, width=HD):
            t = w_pool.tile([128, CT, width], BF16, name=name, tag=name)
            eng.dma_start(
                out=t, in_=dram[:, :].rearrange("(ct p) h -> p ct h", p=128))
            wtiles[name] = t

        def dma_x(qb, eng):
            qs = slice(qb * 512, (qb + 1) * 512)
            eng.dma_start(
                out=xq_t[qb],
                in_=xT[:, qs].rearrange("(ct p) n -> p ct n", p=128))

        # qb0 halves first on sync; wk then wq halves first on scalar
        for h in range(2):
            cs = slice(h * 512, h * 512 + 512)
            nc.sync.dma_start(
                out=xq0_t[h],
                in_=xT[cs, 0:512].rearrange("(ct p) n -> p ct n", p=128))
        wkq_tile = w_pool.tile([128, CT, 2 * HD], BF16, name="wkq",
                               tag="wkq")
        wtiles["wkq"] = wkq_tile
        for h in range(2):
            ws = slice(h * HD, (h + 1) * HD)
            nc.scalar.dma_start(
                out=wkq_tile[:, :, ws],
                in_=wkq[:, ws].rearrange("(ct p) h -> p ct h", p=128))
        dma_x(1, nc.sync)
        dma_w("wv", wv, nc.scalar)
        dma_x(2, nc.sync)
        dma_x(3, nc.scalar)
        wp_full = w_pool.tile([128, HDT, C], BF16, name="wp", tag="wp")
        nc.sync.dma_start(
            out=wp_full, in_=wp[:, :].rearrange("(ht p) c -> p ht c", p=128))

        # preload the exp table set while DMAs land (after the scalar-queue
        # DMA issues so it doesn't delay them)
        warm = const_pool.tile([128, 64], BF16)
        nc.scalar.activation(warm, ones_f, mybir.ActivationFunctionType.Exp,
                             scale=0.0)

        # ---- persistent activations (all bf16) ----
        qT_t = [qk_pool.tile([128, N], BF16, name=f"qT{i}", tag=f"qT{i}")
                for i in range(HDT)]
        kT_t = [qk_pool.tile([128, N], BF16, name=f"kT{i}", tag=f"kT{i}")
                for i in range(HDT)]
        vo_t = [vo_pool.tile([128, HPC * (D + 1)], BF16, name=f"vo{i}",
                             tag=f"vo{i}") for i in range(QT)]
        oht_t = [oht_pool.tile([128, N], BF16, name=f"oht{i}", tag=f"oht{i}")
                 for i in range(HDT)]
        pt_t = [pt_pool.tile([128, 2, 512], BF16, name=f"pt{i}",
                             tag=f"pt{i}") for i in range(QT)]
        yout = [None]  # current output staging tile

        for t in vo_t:
            ones_col = t.rearrange("p (h e) -> p h e", h=HPC)[:, :, D:D + 1]
            nc.gpsimd.tensor_copy(
                ones_col, ones_b[:, 0:HPC].rearrange("p (h o) -> p h o", o=1))

        # ---- work quanta ----
        def q_projqk(ht, qb, dst_t, woff):
            def go():
                w_full = wtiles["wkq"]
                cs = slice(qb * 512, (qb + 1) * 512)
                ps = proj_pool.tile([128, 512], F32, name="proj", tag="proj")
                for ct in range(CT):
                    nc.tensor.matmul(
                        ps,
                        w_full[:, ct, woff + ht * 128:woff + (ht + 1) * 128],
                        xq(qb, ct),
                        start=(ct == 0), stop=(ct == CT - 1))
                nc.vector.tensor_copy(dst_t[ht][:, cs], ps)
            return go, CT * PROJ_MM_NS

        def q_projv(kt):
            def go():
                qbk, off = divmod(kt * 128, 512)
                ks = slice(off, off + 128)
                ps = proj_pool.tile([128, 512], F32, name="proj", tag="proj")
                for ct in range(CT):
                    nc.tensor.matmul(ps[:, 0:HD], xq(qbk, ct)[:, ks],
                                     wtiles["wv"][:, ct, :],
                                     start=(ct == 0), stop=(ct == CT - 1))
                vo_view = vo_t[kt].rearrange("p (h e) -> p h e", h=HPC)
                ps_view = ps[:, 0:HD].rearrange("p (h d) -> p h d", h=HPC)
                nc.vector.tensor_copy(vo_view[:, :, 0:D], ps_view)
            return go, CT * PROJ_MM_NS

        def q_projout(qb, ct):
            def go():
                qs = slice(qb * 512, (qb + 1) * 512)
                cs = slice(ct * 128, (ct + 1) * 128)
                if ct == 0:
                    yout[0] = out_pool.tile([128, CT, 512], BF16, name="yo",
                                            tag="yo")
                ps = proj_pool.tile([128, 512], F32, name="proj", tag="proj")
                for ht in range(HDT):
                    nc.tensor.matmul(
                        ps, wp_full[:, ht, cs], oht_t[ht][:, qs],
                        start=(ht == 0), stop=(ht == HDT - 1))
                nc.vector.tensor_copy(yout[0][:, ct, :], ps)
                if ct == CT - 1:
                    nc.sync.dma_start(
                        out=yT[:, qs].rearrange("(ct p) n -> p ct n", p=128),
                        in_=yout[0])
            return go, HDT * PROJ_MM_NS

        # ---- slot-stream emission ----
        state = {"pe": 0.0, "act": 0.0}
        fifo = []          # [(deadline, go, cost), ...] kept sorted
        done_ids = set()

        def push(deadline, qid, quantum):
            go, cost = quantum
            fifo.append([deadline, qid, go, cost])
            fifo.sort(key=lambda e: e[0])

        def run_item(item):
            _, qid, go, cost = item
            go()
            state["pe"] += cost
            done_ids.add(qid)

        def force_until(g):
            while fifo and fifo[0][0] <= g:
                run_item(fifo.pop(0))

        def budget_drain():
            while fifo and state["pe"] + fifo[0][3] <= state["act"] + AHEAD_NS:
                run_item(fifo.pop(0))

        # prologue projections: kT/qT for (ht0, qb0)
        for dst, woff in ((kT_t, 0), (qT_t, HD)):
            go, cost = q_projqk(0, 0, dst, woff)
            go()
            state["pe"] += cost

        # weave queue: deadlines in global slot units
        for qbk in range(1, QB):
            push(4 * qbk, ("kT", 0, qbk), q_projqk(0, qbk, kT_t, 0))
        for kt in range(QT):
            push(kt + LAG, ("v", kt), q_projv(kt))
        for qb in range(1, QB):
            push(16 * qb, ("qT", 0, qb), q_projqk(0, qb, qT_t, HD))
        for qbk in range(QB):
            push(64 + 4 * qbk - 8, ("kT", 1, qbk), q_projqk(1, qbk, kT_t, 0))
        for qb in range(QB):
            push(64 + 16 * qb, ("qT", 1, qb), q_projqk(1, qb, qT_t, HD))

        iters = [(ht, qb) for ht in range(HDT) for qb in range(QB)]
        ots_by_it = {}
        norm_pending = []
        normb_pending = []

        def emit_s_exp(it, kt):
            ht, qb = iters[it]
            qs = slice(qb * 512, (qb + 1) * 512)
            st = st_pool.tile([128, 2, 512], F32, name="st", tag="st",
                              bufs=2)
            for hp in range(2):
                prow = slice(hp * 64, hp * 64 + 64)
                nc.tensor.matmul(
                    st[:, hp, :],
                    kT_t[ht][prow, kt * 128:(kt + 1) * 128],
                    qT_t[ht][prow, qs])
            nc.scalar.activation(
                pt_t[kt], st, mybir.ActivationFunctionType.Exp, scale=SCALE)
            state["pe"] += SPAIR_NS
            state["act"] += EXP_NS

        def emit_pv(it, kt):
            ht, qb = iters[it]
            if kt == 0:
                ots_by_it[it] = [
                    ot_pool.tile([65, 512], F32, name=f"ot{hp}",
                                 tag=f"ot{hp}", bufs=1)
                    for hp in range(2)]
            ots = ots_by_it[it]
            for hp in range(2):
                h = 2 * ht + hp
                nc.tensor.matmul(
                    ots[hp],
                    vo_t[kt][:, h * (D + 1):(h + 1) * (D + 1)],
                    pt_t[kt][:, hp, :],
                    start=(kt == 0), stop=(kt == QT - 1))
            state["pe"] += 2 * PV_NS
            if kt == QT - 1:
                norm_pending.append(it)

        def emit_norm_a(it):
            # free the PSUM accumulators ASAP: stage O + rowsum to SBUF.
            # rowsum goes to its own partition-0 tile: reciprocal_approx_fast
            # breaks on nonzero base partitions as well as on PSUM reads.
            stgs = []
            ots = ots_by_it.pop(it)
            for hp in range(2):
                stg = small_pool.tile([64, 512], F32, name=f"stg{hp}",
                                      tag=f"stg{hp}")
                nc.vector.tensor_copy(stg, ots[hp][0:64, :])
                sdb = small_pool.tile([1, 512], F32, name=f"sd{hp}",
                                      tag=f"sd{hp}")
                nc.vector.tensor_copy(sdb, ots[hp][64:65, :])
                stgs.append((stg, sdb))
            return stgs

        def emit_norm_b(it, stgs):
            ht, qb = iters[it]
            qs = slice(qb * 512, (qb + 1) * 512)
            for hp in range(2):
                prow = slice(hp * 64, hp * 64 + 64)
                stg, sdb = stgs[hp]
                r32 = small_pool.tile([1, 512], F32, name="r32", tag="r32")
                # approx_fast's bit-trick seed reads garbage through the
                # PSUM port -- it must read SBUF at partition 0 (HW-verified)
                nc.vector.reciprocal_approx_fast(r32, sdb)
                rbf = small_pool.tile([1, 512], BF16, name="rbf", tag="rbf")
                with nc.allow_low_precision(reason="bf16 softmax denom"):
                    nc.vector.tensor_copy(rbf, r32)
                rbt = proj_pool.tile([128, 512], F32, name="proj", tag="proj")
                nc.tensor.matmul(rbt[0:64, :], ones_b[0:1, :], rbf)
                dst = oht_t[ht][prow, qs]
                with nc.allow_low_precision(reason="bf16 attention out"):
                    nc.vector.tensor_mul(dst, stg[0:64, :], rbt[0:64, :])
                if it == NIT - 1 and hp == 1:
                    bt = small_pool.tile([64, 512], F32, name="bt", tag="bt")
                    nc.gpsimd.partition_broadcast(bt, r32)
                    nc.sync.dma_start(out=bdbg[:, :], in_=bt)
            if ht == HDT - 1:
                base = (4 + qb) * 16 + 32
                for ct in range(CT):
                    dl = base + 2 * ct if qb < QB - 1 else 10 ** 6
                    push(dl, ("out", qb, ct), q_projout(qb, ct))

        total_slots = NIT * QT
        for g in range(total_slots + LAG):
            force_until(g)
            if g < total_slots:
                it, kt = divmod(g, QT)
                emit_s_exp(it, kt)
            if norm_pending:
                itn = norm_pending.pop(0)
                normb_pending.append((g + 2, itn, emit_norm_a(itn)))
            if normb_pending and normb_pending[0][0] <= g:
                _, itn, stgs = normb_pending.pop(0)
                emit_norm_b(itn, stgs)
            gpv = g - LAG
            if gpv >= 0:
                itp, ktp = divmod(gpv, QT)
                emit_pv(itp, ktp)
            budget_drain()
        if norm_pending:
            itn = norm_pending.pop(0)
            emit_norm_b(itn, emit_norm_a(itn))
        while normb_pending:
            _, itn, stgs = normb_pending.pop(0)
            emit_norm_b(itn, stgs)
        while fifo:
            run_item(fifo.pop(0))

    nc.finalize()
    return nc


_NC_CACHE = None
TRACE = False
LAST_RESULTS = None


def _get_nc():
    global _NC_CACHE
    if _NC_CACHE is None:
        _NC_CACHE = _build()
    return _NC_CACHE


def kernel(x, w_qkv, w_proj, b_proj):
    global LAST_RESULTS
    import ml_dtypes
    from concourse.bass_utils import run_bass_kernel_spmd

    BF = ml_dtypes.bfloat16
    x = np.asarray(x, dtype=np.float32)
    w_qkv = np.asarray(w_qkv, dtype=np.float32)
    w_proj = np.asarray(w_proj, dtype=np.float32)
    b_proj = np.asarray(b_proj, dtype=np.float32)

    nc = _get_nc()
    xT_b = [np.ascontiguousarray(x[b].T.astype(BF)) for b in range(B)]
    in_maps = []
    for c in range(NCORES):
        b, g = divmod(c, NCORES // B)
        hs = slice(g * HD, (g + 1) * HD)
        wk_g = w_qkv[:, 1 * C:2 * C][:, hs]
        wq_g = w_qkv[:, 0 * C:1 * C][:, hs]
        in_maps.append({
            "xT": xT_b[b],
            "wkq": np.ascontiguousarray(
                np.concatenate([wk_g, wq_g], axis=1).astype(BF)),
            "wv": np.ascontiguousarray(w_qkv[:, 2 * C:3 * C][:, hs].astype(BF)),
            "wp": np.ascontiguousarray(w_proj[g * HD:(g + 1) * HD, :].astype(BF)),
        })
    res = run_bass_kernel_spmd(nc, in_maps, list(range(NCORES)), trace=TRACE)
    LAST_RESULTS = res
    out = np.empty((B, N, C), dtype=np.float32)
    ncb = NCORES // B
    for b in range(B):
        acc = res.results[b * ncb]["yT"].astype(np.float32)
        for g in range(1, ncb):
            acc += res.results[b * ncb + g]["yT"].astype(np.float32)
        out[b] = acc.T + b_proj
    return out


# revision 18
# speedup vs baseline: 1.0538x; 1.0538x over previous
"""Multi-head attention kernel for Trainium2, 8-core tensor/data parallel.

Problem: x[2,2048,1024] -> qkv proj (w_qkv [1024,3072]) -> 16-head attention
         -> out proj (w_proj [1024,1024]) + b_proj.

Sharding: core c handles batch b=c//4 and heads 4*(c%4)..4*(c%4)+4.
Each core computes a partial output Y^T = w_proj_rows^T @ OH (its 4 heads'
contribution, transposed); the host sums the 4 partials per batch,
transposes, and adds the bias.

Schedule: the kernel is ACT-engine bound (softmax exp is ~147us of scalar
engine time per core and exp only runs there), so everything is organized
as a single slot stream paced by the exp chain.  Each slot emits one
S^T matmul pair (row-tiled K=64 halves that run concurrently on the PE
array), the exp for that tile, and the PV matmul pair for the slot LAG
positions earlier; Q/K/V projections and the output projection are woven
into the remaining PE capacity between slots using a ns-budget model so
the PE queue never runs ahead of the ACT queue (which would stall exp).
All data is bf16 (inputs, weights, activations, output partials); PSUM
accumulation stays fp32.  Softmax skips max-subtraction (scores are
~N(0,1) after the 1/sqrt(D) scale) and folds the row-sum into the PV
matmul via an appended ones-column on V; denominators use the fast
approximate reciprocal (~18 bits, plenty for this tolerance).
"""

from contextlib import ExitStack

import numpy as np

import concourse.bass as bass
import concourse.mybir as mybir
from concourse import bacc, tile

B, N, C, H = 2, 2048, 1024, 16
D = C // H            # 64 head dim
SCALE = float(D) ** -0.5
HPC = 4               # heads per core
HD = HPC * D          # 256 head-dim columns per core
NCORES = 8

F32 = mybir.dt.float32
F32R = mybir.dt.float32r
BF16 = mybir.dt.bfloat16

QT = N // 128         # 16 query/key 128-tiles
CT = C // 128         # 8 channel 128-tiles
QB = N // 512         # 4 query 512-blocks
HDT = HD // 128       # 2 head-pair tiles (2 heads of 64 each)

NIT = HDT * QB        # 8 attention iterations (ht-major)
LAG = 8               # PV trails its exp by this many slots

# empirical effective PE costs (ns), from HW trace at nominal clock
SPAIR_NS = 390.0      # row-tiled S matmul pair (2x [64]x512)
PV_NS = 225.0         # one PV matmul, ap=512
PROJ_MM_NS = 240.0    # projection matmul, ap=512
RB_NS = 250.0         # ones-broadcast matmul
EXP_NS = 1147.0       # ACT ns per [128,2,512] exp
AHEAD_NS = 1200.0     # how far PE emission may run ahead of ACT


def _build():
    nc = bacc.Bacc(None)
    xT = nc.declare_dram_parameter("xT", [C, N], BF16, isOutput=False)
    wkq = nc.declare_dram_parameter("wkq", [C, 2 * HD], BF16,
                                    isOutput=False)
    wv = nc.declare_dram_parameter("wv", [C, HD], BF16, isOutput=False)
    wp = nc.declare_dram_parameter("wp", [HD, C], BF16, isOutput=False)
    yT = nc.declare_dram_parameter("yT", [C, N], BF16, isOutput=True)

    with tile.TileContext(nc) as tc, ExitStack() as ctx:
        const_pool = ctx.enter_context(tc.tile_pool(name="const", bufs=1))
        w_pool = ctx.enter_context(tc.tile_pool(name="w", bufs=1))
        x_pool = ctx.enter_context(tc.tile_pool(name="x", bufs=1))
        qk_pool = ctx.enter_context(tc.tile_pool(name="qk", bufs=1))
        vo_pool = ctx.enter_context(tc.tile_pool(name="vo", bufs=1))
        oht_pool = ctx.enter_context(tc.tile_pool(name="oht", bufs=1))
        pt_pool = ctx.enter_context(tc.tile_pool(name="pt", bufs=1))
        small_pool = ctx.enter_context(tc.tile_pool(name="small", bufs=2))
        out_pool = ctx.enter_context(tc.tile_pool(name="out", bufs=2))
        st_pool = ctx.enter_context(
            tc.tile_pool(name="ps_st", bufs=2, space="PSUM"))
        ot_pool = ctx.enter_context(
            tc.tile_pool(name="ps_ot", bufs=1, space="PSUM"))
        proj_pool = ctx.enter_context(
            tc.tile_pool(name="ps_proj", bufs=2, space="PSUM"))

        ones_f = const_pool.tile([128, 64], F32)
        nc.vector.memset(ones_f, 1.0)
        ones_b = const_pool.tile([128, 64], BF16)
        nc.vector.memset(ones_b, 1.0)

        # ---- input DMAs, ordered by first use ----
        xq0_t = [x_pool.tile([128, CT // 2, 512], BF16, name=f"xq0{h}",
                             tag=f"xq0{h}") for h in range(2)]
        xq_t = [None] + [x_pool.tile([128, CT, 512], BF16, name=f"xq{qb}",
                                     tag=f"xq{qb}") for qb in range(1, QB)]
        wtiles = {}

        def xq(qb, ct):
            if qb == 0:
                return xq0_t[ct // (CT // 2)][:, ct % (CT // 2), :]
            return xq_t[qb][:, ct, :]

        # split input DMAs across the two hw DMA-gen engines (sync + scalar)
        # so transfers run on two queues in parallel; ACT is idle at startup
        def dma_w(name, dram, eng, width=HD):
            t = w_pool.tile([128, CT, width], BF16, name=name, tag=name)
            eng.dma_start(
                out=t, in_=dram[:, :].rearrange("(ct p) h -> p ct h", p=128))
            wtiles[name] = t

        def dma_x(qb, eng):
            qs = slice(qb * 512, (qb + 1) * 512)
            eng.dma_start(
                out=xq_t[qb],
                in_=xT[:, qs].rearrange("(ct p) n -> p ct n", p=128))

        # qb0 halves first on sync; wk then wq halves first on scalar
        for h in range(2):
            cs = slice(h * 512, h * 512 + 512)
            nc.sync.dma_start(
                out=xq0_t[h],
                in_=xT[cs, 0:512].rearrange("(ct p) n -> p ct n", p=128))
        wkq_tile = w_pool.tile([128, CT, 2 * HD], BF16, name="wkq",
                               tag="wkq")
        wtiles["wkq"] = wkq_tile
        for h in range(2):
            ws = slice(h * HD, (h + 1) * HD)
            nc.scalar.dma_start(
                out=wkq_tile[:, :, ws],
                in_=wkq[:, ws].rearrange("(ct p) h -> p ct h", p=128))
        dma_x(1, nc.sync)
        dma_w("wv", wv, nc.scalar)
        dma_x(2, nc.sync)
        dma_x(3, nc.scalar)
        wp_full = w_pool.tile([128, HDT, C], BF16, name="wp", tag="wp")
        nc.sync.dma_start(
            out=wp_full, in_=wp[:, :].rearrange("(ht p) c -> p ht c", p=128))

        # preload the exp table set while DMAs land (after the scalar-queue
        # DMA issues so it doesn't delay them)
        warm = const_pool.tile([128, 64], BF16)
        nc.scalar.activation(warm, ones_f, mybir.ActivationFunctionType.Exp,
                             scale=0.0)

        # ---- persistent activations (all bf16) ----
        qT_t = [qk_pool.tile([128, N], BF16, name=f"qT{i}", tag=f"qT{i}")
                for i in range(HDT)]
        kT_t = [qk_pool.tile([128, N], BF16, name=f"kT{i}", tag=f"kT{i}")
                for i in range(HDT)]
        vo_t = [vo_pool.tile([128, HPC * (D + 1)], BF16, name=f"vo{i}",
                             tag=f"vo{i}") for i in range(QT)]
        oht_t = [oht_pool.tile([128, N], BF16, name=f"oht{i}", tag=f"oht{i}")
                 for i in range(HDT)]
        pt_t = [pt_pool.tile([128, 2, 512], BF16, name=f"pt{i}",
                             tag=f"pt{i}") for i in range(QT)]
        yout = [None]  # current output staging tile

        for t in vo_t:
            ones_col = t.rearrange("p (h e) -> p h e", h=HPC)[:, :, D:D + 1]
            nc.gpsimd.tensor_copy(
                ones_col, ones_b[:, 0:HPC].rearrange("p (h o) -> p h o", o=1))

        # ---- work quanta ----
        def q_projqk(ht, qb, dst_t, woff):
            def go():
                w_full = wtiles["wkq"]
                cs = slice(qb * 512, (qb + 1) * 512)
                ps = proj_pool.tile([128, 512], F32, name="proj", tag="proj")
                for ct in range(CT):
                    nc.tensor.matmul(
                        ps,
                        w_full[:, ct, woff + ht * 128:woff + (ht + 1) * 128],
                        xq(qb, ct),
                        start=(ct == 0), stop=(ct == CT - 1))
                nc.vector.tensor_copy(dst_t[ht][:, cs], ps)
            return go, CT * PROJ_MM_NS

        def q_projv(kt):
            def go():
                qbk, off = divmod(kt * 128, 512)
                ks = slice(off, off + 128)
                ps = proj_pool.tile([128, 512], F32, name="proj", tag="proj")
                for ct in range(CT):
                    nc.tensor.matmul(ps[:, 0:HD], xq(qbk, ct)[:, ks],
                                     wtiles["wv"][:, ct, :],
                                     start=(ct == 0), stop=(ct == CT - 1))
                vo_view = vo_t[kt].rearrange("p (h e) -> p h e", h=HPC)
                ps_view = ps[:, 0:HD].rearrange("p (h d) -> p h d", h=HPC)
                nc.vector.tensor_copy(vo_view[:, :, 0:D], ps_view)
            return go, CT * PROJ_MM_NS

        def q_projout(qb, ct):
            def go():
                qs = slice(qb * 512, (qb + 1) * 512)
                cs = slice(ct * 128, (ct + 1) * 128)
                if ct == 0:
                    yout[0] = out_pool.tile([128, CT, 512], BF16, name="yo",
                                            tag="yo")
                ps = proj_pool.tile([128, 512], F32, name="proj", tag="proj")
                for ht in range(HDT):
                    nc.tensor.matmul(
                        ps, wp_full[:, ht, cs], oht_t[ht][:, qs],
                        start=(ht == 0), stop=(ht == HDT - 1))
                nc.vector.tensor_copy(yout[0][:, ct, :], ps)
                if ct == CT - 1:
                    nc.sync.dma_start(
                        out=yT[:, qs].rearrange("(ct p) n -> p ct n", p=128),
                        in_=yout[0])
            return go, HDT * PROJ_MM_NS

        # ---- slot-stream emission ----
        state = {"pe": 0.0, "act": 0.0}
        fifo = []          # [(deadline, go, cost), ...] kept sorted
        done_ids = set()

        def push(deadline, qid, quantum):
            go, cost = quantum
            fifo.append([deadline, qid, go, cost])
            fifo.sort(key=lambda e: e[0])

        def run_item(item):
            _, qid, go, cost = item
            go()
            state["pe"] += cost
            done_ids.add(qid)

        def force_until(g):
            while fifo and fifo[0][0] <= g:
                run_item(fifo.pop(0))

        def budget_drain():
            while fifo and state["pe"] + fifo[0][3] <= state["act"] + AHEAD_NS:
                run_item(fifo.pop(0))

        # prologue projections: kT/qT for (ht0, qb0)
        for dst, woff in ((kT_t, 0), (qT_t, HD)):
            go, cost = q_projqk(0, 0, dst, woff)
            go()
            state["pe"] += cost

        # weave queue: deadlines in global slot units
        for qbk in range(1, QB):
            push(4 * qbk, ("kT", 0, qbk), q_projqk(0, qbk, kT_t, 0))
        for kt in range(QT):
            push(kt + LAG, ("v", kt), q_projv(kt))
        for qb in range(1, QB):
            push(16 * qb, ("qT", 0, qb), q_projqk(0, qb, qT_t, HD))
        for qbk in range(QB):
            push(64 + 4 * qbk - 8, ("kT", 1, qbk), q_projqk(1, qbk, kT_t, 0))
        for qb in range(QB):
            push(64 + 16 * qb, ("qT", 1, qb), q_projqk(1, qb, qT_t, HD))

        iters = [(ht, qb) for ht in range(HDT) for qb in range(QB)]
        ots_by_it = {}
        norm_pending = []
        normb_pending = []

        def emit_s_exp(it, kt):
            ht, qb = iters[it]
            qs = slice(qb * 512, (qb + 1) * 512)
            st = st_pool.tile([128, 2, 512], F32, name="st", tag="st",
                              bufs=2)
            for hp in range(2):
                prow = slice(hp * 64, hp * 64 + 64)
                nc.tensor.matmul(
                    st[:, hp, :],
                    kT_t[ht][prow, kt * 128:(kt + 1) * 128],
                    qT_t[ht][prow, qs])
            nc.scalar.activation(
                pt_t[kt], st, mybir.ActivationFunctionType.Exp, scale=SCALE)
            state["pe"] += SPAIR_NS
            state["act"] += EXP_NS

        def emit_pv(it, kt):
            ht, qb = iters[it]
            if kt == 0:
                ots_by_it[it] = [
                    ot_pool.tile([65, 512], F32, name=f"ot{hp}",
                                 tag=f"ot{hp}", bufs=1)
                    for hp in range(2)]
            ots = ots_by_it[it]
            for hp in range(2):
                h = 2 * ht + hp
                nc.tensor.matmul(
                    ots[hp],
                    vo_t[kt][:, h * (D + 1):(h + 1) * (D + 1)],
                    pt_t[kt][:, hp, :],
                    start=(kt == 0), stop=(kt == QT - 1))
            state["pe"] += 2 * PV_NS
            if kt == QT - 1:
                norm_pending.append(it)

        def emit_norm_a(it):
            # free the PSUM accumulators ASAP: stage O + rowsum to SBUF.
            # rowsum goes to its own partition-0 tile: reciprocal_approx_fast
            # breaks on nonzero base partitions as well as on PSUM reads.
            stgs = []
            ots = ots_by_it.pop(it)
            for hp in range(2):
                stg = small_pool.tile([64, 512], F32, name=f"stg{hp}",
                                      tag=f"stg{hp}")
                nc.vector.tensor_copy(stg, ots[hp][0:64, :])
                sdb = small_pool.tile([1, 512], F32, name=f"sd{hp}",
                                      tag=f"sd{hp}")
                nc.vector.tensor_copy(sdb, ots[hp][64:65, :])
                stgs.append((stg, sdb))
            return stgs

        def emit_norm_b(it, stgs):
            ht, qb = iters[it]
            qs = slice(qb * 512, (qb + 1) * 512)
            for hp in range(2):
                prow = slice(hp * 64, hp * 64 + 64)
                stg, sdb = stgs[hp]
                r32 = small_pool.tile([1, 512], F32, name="r32", tag="r32")
                # approx_fast's bit-trick seed reads garbage through the
                # PSUM port -- it must read SBUF at partition 0 (HW-verified)
                nc.vector.reciprocal_approx_fast(r32, sdb)
                # replicate 1/rowsum across partitions on the idle GPSIMD
                # engine (HW-verified bit-faithful); keeps norm off the PE
                rb = small_pool.tile([64, 512], F32, name="rb", tag="rb")
                nc.gpsimd.partition_broadcast(rb, r32)
                dst = oht_t[ht][prow, qs]
                with nc.allow_low_precision(reason="bf16 attention out"):
                    nc.vector.tensor_mul(dst, stg[0:64, :], rb)
            if ht == HDT - 1:
                base = (4 + qb) * 16 + 32
                for ct in range(CT):
                    dl = base + 2 * ct if qb < QB - 1 else 10 ** 6
                    push(dl, ("out", qb, ct), q_projout(qb, ct))

        total_slots = NIT * QT
        for g in range(total_slots + LAG):
            force_until(g)
            if g < total_slots:
                it, kt = divmod(g, QT)
                emit_s_exp(it, kt)
            if norm_pending:
                itn = norm_pending.pop(0)
                normb_pending.append((g + 2, itn, emit_norm_a(itn)))
            if normb_pending and normb_pending[0][0] <= g:
                _, itn, stgs = normb_pending.pop(0)
                emit_norm_b(itn, stgs)
            gpv = g - LAG
            if gpv >= 0:
                itp, ktp = divmod(gpv, QT)
                emit_pv(itp, ktp)
            budget_drain()
        if norm_pending:
            itn = norm_pending.pop(0)
            emit_norm_b(itn, emit_norm_a(itn))
        while normb_pending:
            _, itn, stgs = normb_pending.pop(0)
            emit_norm_b(itn, stgs)
        while fifo:
            run_item(fifo.pop(0))

    nc.finalize()
    return nc


_NC_CACHE = None
TRACE = False
LAST_RESULTS = None


def _get_nc():
    global _NC_CACHE
    if _NC_CACHE is None:
        _NC_CACHE = _build()
    return _NC_CACHE


def kernel(x, w_qkv, w_proj, b_proj):
    global LAST_RESULTS
    import ml_dtypes
    from concourse.bass_utils import run_bass_kernel_spmd

    BF = ml_dtypes.bfloat16
    x = np.asarray(x, dtype=np.float32)
    w_qkv = np.asarray(w_qkv, dtype=np.float32)
    w_proj = np.asarray(w_proj, dtype=np.float32)
    b_proj = np.asarray(b_proj, dtype=np.float32)

    nc = _get_nc()
    xT_b = [np.ascontiguousarray(x[b].T.astype(BF)) for b in range(B)]
    in_maps = []
    for c in range(NCORES):
        b, g = divmod(c, NCORES // B)
        hs = slice(g * HD, (g + 1) * HD)
        wk_g = w_qkv[:, 1 * C:2 * C][:, hs]
        wq_g = w_qkv[:, 0 * C:1 * C][:, hs]
        in_maps.append({
            "xT": xT_b[b],
            "wkq": np.ascontiguousarray(
                np.concatenate([wk_g, wq_g], axis=1).astype(BF)),
            "wv": np.ascontiguousarray(w_qkv[:, 2 * C:3 * C][:, hs].astype(BF)),
            "wp": np.ascontiguousarray(w_proj[g * HD:(g + 1) * HD, :].astype(BF)),
        })
    res = run_bass_kernel_spmd(nc, in_maps, list(range(NCORES)), trace=TRACE)
    LAST_RESULTS = res
    out = np.empty((B, N, C), dtype=np.float32)
    ncb = NCORES // B
    for b in range(B):
        acc = res.results[b * ncb]["yT"].astype(np.float32)
        for g in range(1, ncb):
            acc += res.results[b * ncb + g]["yT"].astype(np.float32)
        out[b] = acc.T + b_proj
    return out


# revision 19
# speedup vs baseline: 1.2148x; 1.1528x over previous
"""Multi-head attention kernel for Trainium2, 8-core tensor/data parallel.

Problem: x[2,2048,1024] -> qkv proj (w_qkv [1024,3072]) -> 16-head attention
         -> out proj (w_proj [1024,1024]) + b_proj.

Sharding: core c handles batch b=c//4 and heads 4*(c%4)..4*(c%4)+4.
Each core computes a partial output Y^T = w_proj_rows^T @ OH (its 4 heads'
contribution, transposed); the host sums the 4 partials per batch,
transposes, and adds the bias.

Schedule: the kernel is ACT-engine bound (softmax exp is ~147us of scalar
engine time per core and exp only runs there), so everything is organized
as a single slot stream paced by the exp chain.  Each slot emits one
S^T matmul pair (row-tiled K=64 halves that run concurrently on the PE
array), the exp for that tile, and the PV matmul pair for the slot LAG
positions earlier; Q/K/V projections and the output projection are woven
into the remaining PE capacity between slots using a ns-budget model so
the PE queue never runs ahead of the ACT queue (which would stall exp).
All data is bf16 (inputs, weights, activations, output partials); PSUM
accumulation stays fp32.  Softmax skips max-subtraction (scores are
~N(0,1) after the 1/sqrt(D) scale) and folds the row-sum into the PV
matmul via an appended ones-column on V; denominators use the fast
approximate reciprocal (~18 bits, plenty for this tolerance).
"""

from contextlib import ExitStack

import numpy as np

import concourse.bass as bass
import concourse.mybir as mybir
from concourse import bacc, tile

B, N, C, H = 2, 2048, 1024, 16
D = C // H            # 64 head dim
SCALE = float(D) ** -0.5
HPC = 4               # heads per core
HD = HPC * D          # 256 head-dim columns per core
NCORES = 8

F32 = mybir.dt.float32
F32R = mybir.dt.float32r
BF16 = mybir.dt.bfloat16

QT = N // 128         # 16 query/key 128-tiles
CT = C // 128         # 8 channel 128-tiles
QB = N // 512         # 4 query 512-blocks
HDT = HD // 128       # 2 head-pair tiles (2 heads of 64 each)

NIT = HDT * QB        # 8 attention iterations (ht-major)
LAG = 8               # PV trails its exp by this many slots

# empirical effective PE costs (ns), from HW trace at nominal clock
SPAIR_NS = 390.0      # row-tiled S matmul pair (2x [64]x512)
PV_NS = 225.0         # one PV matmul, ap=512
PROJ_MM_NS = 240.0    # projection matmul, ap=512
RB_NS = 250.0         # ones-broadcast matmul
EXP_NS = 1147.0       # ACT ns per [128,2,512] exp
AHEAD_NS = 2000.0     # how far PE emission may run ahead of ACT


def _build():
    nc = bacc.Bacc(None)
    xT = nc.declare_dram_parameter("xT", [C, N], BF16, isOutput=False)
    wkq = nc.declare_dram_parameter("wkq", [C, 2 * HD], BF16,
                                    isOutput=False)
    wv = nc.declare_dram_parameter("wv", [C, HD], BF16, isOutput=False)
    wp = nc.declare_dram_parameter("wp", [HD, C], BF16, isOutput=False)
    yT = nc.declare_dram_parameter("yT", [C, N], BF16, isOutput=True)

    with tile.TileContext(nc) as tc, ExitStack() as ctx:
        const_pool = ctx.enter_context(tc.tile_pool(name="const", bufs=1))
        w_pool = ctx.enter_context(tc.tile_pool(name="w", bufs=1))
        x_pool = ctx.enter_context(tc.tile_pool(name="x", bufs=1))
        qk_pool = ctx.enter_context(tc.tile_pool(name="qk", bufs=1))
        vo_pool = ctx.enter_context(tc.tile_pool(name="vo", bufs=1))
        oht_pool = ctx.enter_context(tc.tile_pool(name="oht", bufs=1))
        pt_pool = ctx.enter_context(tc.tile_pool(name="pt", bufs=1))
        small_pool = ctx.enter_context(tc.tile_pool(name="small", bufs=2))
        out_pool = ctx.enter_context(tc.tile_pool(name="out", bufs=2))
        st_pool = ctx.enter_context(
            tc.tile_pool(name="ps_st", bufs=2, space="PSUM"))
        ot_pool = ctx.enter_context(
            tc.tile_pool(name="ps_ot", bufs=1, space="PSUM"))
        proj_pool = ctx.enter_context(
            tc.tile_pool(name="ps_proj", bufs=2, space="PSUM"))

        ones_f = const_pool.tile([128, 64], F32)
        nc.vector.memset(ones_f, 1.0)
        ones_b = const_pool.tile([128, 64], BF16)
        nc.vector.memset(ones_b, 1.0)

        # ---- input DMAs, ordered by first use ----
        xq0_t = [x_pool.tile([128, CT // 2, 512], BF16, name=f"xq0{h}",
                             tag=f"xq0{h}") for h in range(2)]
        xq_t = [None] + [x_pool.tile([128, CT, 512], BF16, name=f"xq{qb}",
                                     tag=f"xq{qb}") for qb in range(1, QB)]
        wtiles = {}

        def xq(qb, ct):
            if qb == 0:
                return xq0_t[ct // (CT // 2)][:, ct % (CT // 2), :]
            return xq_t[qb][:, ct, :]

        # split input DMAs across the two hw DMA-gen engines (sync + scalar)
        # so transfers run on two queues in parallel; ACT is idle at startup
        def dma_w(name, dram, eng, width=HD):
            t = w_pool.tile([128, CT, width], BF16, name=name, tag=name)
            eng.dma_start(
                out=t, in_=dram[:, :].rearrange("(ct p) h -> p ct h", p=128))
            wtiles[name] = t

        def dma_x(qb, eng):
            qs = slice(qb * 512, (qb + 1) * 512)
            eng.dma_start(
                out=xq_t[qb],
                in_=xT[:, qs].rearrange("(ct p) n -> p ct n", p=128))

        # qb0 halves first on sync; wk then wq halves first on scalar
        for h in range(2):
            cs = slice(h * 512, h * 512 + 512)
            nc.sync.dma_start(
                out=xq0_t[h],
                in_=xT[cs, 0:512].rearrange("(ct p) n -> p ct n", p=128))
        wkq_tile = w_pool.tile([128, CT, 2 * HD], BF16, name="wkq",
                               tag="wkq")
        wtiles["wkq"] = wkq_tile
        for h in range(2):
            ws = slice(h * HD, (h + 1) * HD)
            nc.scalar.dma_start(
                out=wkq_tile[:, :, ws],
                in_=wkq[:, ws].rearrange("(ct p) h -> p ct h", p=128))
        dma_x(1, nc.sync)
        dma_w("wv", wv, nc.scalar)
        dma_x(2, nc.sync)
        dma_x(3, nc.scalar)
        wp_full = w_pool.tile([128, HDT, C], BF16, name="wp", tag="wp")
        nc.sync.dma_start(
            out=wp_full, in_=wp[:, :].rearrange("(ht p) c -> p ht c", p=128))

        # preload the exp table set while DMAs land (after the scalar-queue
        # DMA issues so it doesn't delay them)
        warm = const_pool.tile([128, 64], BF16)
        nc.scalar.activation(warm, ones_f, mybir.ActivationFunctionType.Exp,
                             scale=0.0)

        # ---- persistent activations (all bf16) ----
        qT_t = [qk_pool.tile([128, N], BF16, name=f"qT{i}", tag=f"qT{i}")
                for i in range(HDT)]
        kT_t = [qk_pool.tile([128, N], BF16, name=f"kT{i}", tag=f"kT{i}")
                for i in range(HDT)]
        vo_t = [vo_pool.tile([128, HPC * (D + 1)], BF16, name=f"vo{i}",
                             tag=f"vo{i}") for i in range(QT)]
        oht_t = [oht_pool.tile([128, N], BF16, name=f"oht{i}", tag=f"oht{i}")
                 for i in range(HDT)]
        pt_t = [pt_pool.tile([128, 2, 512], BF16, name=f"pt{i}",
                             tag=f"pt{i}") for i in range(QT)]
        yout = [None]  # current output staging tile

        for t in vo_t:
            ones_col = t.rearrange("p (h e) -> p h e", h=HPC)[:, :, D:D + 1]
            nc.gpsimd.tensor_copy(
                ones_col, ones_b[:, 0:HPC].rearrange("p (h o) -> p h o", o=1))

        # ---- work quanta ----
        def q_projqk(ht, qb, dst_t, woff):
            def go():
                w_full = wtiles["wkq"]
                cs = slice(qb * 512, (qb + 1) * 512)
                ps = proj_pool.tile([128, 512], F32, name="proj", tag="proj")
                for ct in range(CT):
                    nc.tensor.matmul(
                        ps,
                        w_full[:, ct, woff + ht * 128:woff + (ht + 1) * 128],
                        xq(qb, ct),
                        start=(ct == 0), stop=(ct == CT - 1))
                nc.vector.tensor_copy(dst_t[ht][:, cs], ps)
            return go, CT * PROJ_MM_NS

        def q_projv(kt):
            def go():
                qbk, off = divmod(kt * 128, 512)
                ks = slice(off, off + 128)
                ps = proj_pool.tile([128, 512], F32, name="proj", tag="proj")
                for ct in range(CT):
                    nc.tensor.matmul(ps[:, 0:HD], xq(qbk, ct)[:, ks],
                                     wtiles["wv"][:, ct, :],
                                     start=(ct == 0), stop=(ct == CT - 1))
                vo_view = vo_t[kt].rearrange("p (h e) -> p h e", h=HPC)
                ps_view = ps[:, 0:HD].rearrange("p (h d) -> p h d", h=HPC)
                nc.vector.tensor_copy(vo_view[:, :, 0:D], ps_view)
            return go, CT * PROJ_MM_NS

        def q_projout(qb, ct):
            def go():
                qs = slice(qb * 512, (qb + 1) * 512)
                cs = slice(ct * 128, (ct + 1) * 128)
                if ct == 0:
                    yout[0] = out_pool.tile([128, CT, 512], BF16, name="yo",
                                            tag="yo")
                ps = proj_pool.tile([128, 512], F32, name="proj", tag="proj")
                for ht in range(HDT):
                    nc.tensor.matmul(
                        ps, wp_full[:, ht, cs], oht_t[ht][:, qs],
                        start=(ht == 0), stop=(ht == HDT - 1))
                nc.vector.tensor_copy(yout[0][:, ct, :], ps)
                if ct == CT - 1:
                    nc.sync.dma_start(
                        out=yT[:, qs].rearrange("(ct p) n -> p ct n", p=128),
                        in_=yout[0])
            return go, HDT * PROJ_MM_NS

        # ---- slot-stream emission ----
        state = {"pe": 0.0, "act": 0.0}
        fifo = []          # [(deadline, go, cost), ...] kept sorted
        done_ids = set()

        def push(deadline, qid, quantum):
            go, cost = quantum
            fifo.append([deadline, qid, go, cost])
            fifo.sort(key=lambda e: e[0])

        def run_item(item):
            _, qid, go, cost = item
            go()
            state["pe"] += cost
            done_ids.add(qid)

        def force_until(g):
            while fifo and fifo[0][0] <= g:
                run_item(fifo.pop(0))

        def budget_drain():
            while fifo and state["pe"] + fifo[0][3] <= state["act"] + AHEAD_NS:
                run_item(fifo.pop(0))

        # prologue projections for (ht0, qb0): kT cols 0-255 first (covers
        # kt0/kt1), then full qT, then the kT remainder -- the first S pair
        # only needs the narrow kT chunk plus qT
        def projqk_chunk(dst_t, woff, lo, hi):
            w_full = wtiles["wkq"]
            ps = proj_pool.tile([128, 512], F32, name="proj", tag="proj")
            for ct in range(CT):
                nc.tensor.matmul(
                    ps[:, 0:hi - lo],
                    w_full[:, ct, woff:woff + 128],
                    xq(0, ct)[:, lo:hi],
                    start=(ct == 0), stop=(ct == CT - 1))
            nc.vector.tensor_copy(dst_t[0][:, lo:hi], ps[:, 0:hi - lo])
            state["pe"] += CT * PROJ_MM_NS * (hi - lo) / 512.0

        projqk_chunk(kT_t, 0, 0, 256)
        projqk_chunk(qT_t, HD, 0, 512)
        projqk_chunk(kT_t, 0, 256, 512)

        # weave queue: deadlines in global slot units
        for qbk in range(1, QB):
            push(4 * qbk, ("kT", 0, qbk), q_projqk(0, qbk, kT_t, 0))
        for kt in range(QT):
            push(kt + LAG, ("v", kt), q_projv(kt))
        for qb in range(1, QB):
            push(16 * qb, ("qT", 0, qb), q_projqk(0, qb, qT_t, HD))
        for qbk in range(QB):
            push(64 + 4 * qbk - 8, ("kT", 1, qbk), q_projqk(1, qbk, kT_t, 0))
        for qb in range(QB):
            push(64 + 16 * qb, ("qT", 1, qb), q_projqk(1, qb, qT_t, HD))

        iters = [(ht, qb) for ht in range(HDT) for qb in range(QB)]
        ots_by_it = {}
        norm_pending = []
        normb_pending = []
        pv_sched = {}
        for it in range(NIT):
            lag = 6 if it == NIT - 1 else LAG
            for kt in range(QT):
                pv_sched.setdefault(it * QT + kt + lag, []).append((it, kt))

        def emit_s_exp(it, kt):
            ht, qb = iters[it]
            qs = slice(qb * 512, (qb + 1) * 512)
            st = st_pool.tile([128, 2, 512], F32, name="st", tag="st",
                              bufs=2)
            for hp in range(2):
                prow = slice(hp * 64, hp * 64 + 64)
                nc.tensor.matmul(
                    st[:, hp, :],
                    kT_t[ht][prow, kt * 128:(kt + 1) * 128],
                    qT_t[ht][prow, qs])
            nc.scalar.activation(
                pt_t[kt], st, mybir.ActivationFunctionType.Exp, scale=SCALE)
            state["pe"] += SPAIR_NS
            state["act"] += EXP_NS

        def emit_pv(it, kt):
            ht, qb = iters[it]
            if kt == 0:
                ots_by_it[it] = [
                    ot_pool.tile([65, 512], F32, name=f"ot{hp}",
                                 tag=f"ot{hp}", bufs=1)
                    for hp in range(2)]
            ots = ots_by_it[it]
            for hp in range(2):
                h = 2 * ht + hp
                nc.tensor.matmul(
                    ots[hp],
                    vo_t[kt][:, h * (D + 1):(h + 1) * (D + 1)],
                    pt_t[kt][:, hp, :],
                    start=(kt == 0), stop=(kt == QT - 1))
            state["pe"] += 2 * PV_NS
            if kt == QT - 1:
                norm_pending.append(it)

        def emit_norm_a(it):
            # free the PSUM accumulators ASAP: stage O + rowsum to SBUF.
            # rowsum goes to its own partition-0 tile: reciprocal_approx_fast
            # breaks on nonzero base partitions as well as on PSUM reads.
            stgs = []
            ots = ots_by_it.pop(it)
            for hp in range(2):
                stg = small_pool.tile([64, 512], F32, name=f"stg{hp}",
                                      tag=f"stg{hp}")
                nc.vector.tensor_copy(stg, ots[hp][0:64, :])
                sdb = small_pool.tile([1, 512], F32, name=f"sd{hp}",
                                      tag=f"sd{hp}")
                nc.vector.tensor_copy(sdb, ots[hp][64:65, :])
                stgs.append((stg, sdb))
            return stgs

        def emit_norm_b(it, stgs):
            ht, qb = iters[it]
            qs = slice(qb * 512, (qb + 1) * 512)
            for hp in range(2):
                prow = slice(hp * 64, hp * 64 + 64)
                stg, sdb = stgs[hp]
                r32 = small_pool.tile([1, 512], F32, name="r32", tag="r32")
                # approx_fast's bit-trick seed reads garbage through the
                # PSUM port -- it must read SBUF at partition 0 (HW-verified)
                nc.vector.reciprocal_approx_fast(r32, sdb)
                # replicate 1/rowsum across partitions on the idle GPSIMD
                # engine (HW-verified bit-faithful); keeps norm off the PE
                rb = small_pool.tile([64, 512], F32, name="rb", tag="rb")
                nc.gpsimd.partition_broadcast(rb, r32)
                dst = oht_t[ht][prow, qs]
                with nc.allow_low_precision(reason="bf16 attention out"):
                    nc.vector.tensor_mul(dst, stg[0:64, :], rb)
            if ht == HDT - 1:
                base = (4 + qb) * 16 + 32
                for ct in range(CT):
                    dl = base + 2 * ct if qb < QB - 1 else 10 ** 6
                    push(dl, ("out", qb, ct), q_projout(qb, ct))

        total_slots = NIT * QT
        for g in range(total_slots + LAG):
            force_until(g)
            if g < total_slots:
                it, kt = divmod(g, QT)
                emit_s_exp(it, kt)
            if norm_pending:
                itn = norm_pending.pop(0)
                normb_pending.append((g + 2, itn, emit_norm_a(itn)))
            if normb_pending and normb_pending[0][0] <= g:
                _, itn, stgs = normb_pending.pop(0)
                emit_norm_b(itn, stgs)
            for gpv in pv_sched.pop(g, ()):
                emit_pv(*gpv)
            budget_drain()
        if norm_pending:
            itn = norm_pending.pop(0)
            emit_norm_b(itn, emit_norm_a(itn))
        while normb_pending:
            _, itn, stgs = normb_pending.pop(0)
            emit_norm_b(itn, stgs)
        while fifo:
            run_item(fifo.pop(0))

    nc.finalize()
    return nc


_NC_CACHE = None
TRACE = False
LAST_RESULTS = None


def _get_nc():
    global _NC_CACHE
    if _NC_CACHE is None:
        _NC_CACHE = _build()
    return _NC_CACHE


def kernel(x, w_qkv, w_proj, b_proj):
    global LAST_RESULTS
    import ml_dtypes
    from concourse.bass_utils import run_bass_kernel_spmd

    BF = ml_dtypes.bfloat16
    x = np.asarray(x, dtype=np.float32)
    w_qkv = np.asarray(w_qkv, dtype=np.float32)
    w_proj = np.asarray(w_proj, dtype=np.float32)
    b_proj = np.asarray(b_proj, dtype=np.float32)

    nc = _get_nc()
    xT_b = [np.ascontiguousarray(x[b].T.astype(BF)) for b in range(B)]
    in_maps = []
    for c in range(NCORES):
        b, g = divmod(c, NCORES // B)
        hs = slice(g * HD, (g + 1) * HD)
        wk_g = w_qkv[:, 1 * C:2 * C][:, hs]
        wq_g = w_qkv[:, 0 * C:1 * C][:, hs]
        in_maps.append({
            "xT": xT_b[b],
            "wkq": np.ascontiguousarray(
                np.concatenate([wk_g, wq_g], axis=1).astype(BF)),
            "wv": np.ascontiguousarray(w_qkv[:, 2 * C:3 * C][:, hs].astype(BF)),
            "wp": np.ascontiguousarray(w_proj[g * HD:(g + 1) * HD, :].astype(BF)),
        })
    res = run_bass_kernel_spmd(nc, in_maps, list(range(NCORES)), trace=TRACE)
    LAST_RESULTS = res
    out = np.empty((B, N, C), dtype=np.float32)
    ncb = NCORES // B
    for b in range(B):
        acc = res.results[b * ncb]["yT"].astype(np.float32)
        for g in range(1, ncb):
            acc += res.results[b * ncb + g]["yT"].astype(np.float32)
        out[b] = acc.T + b_proj
    return out


# revision 20
# speedup vs baseline: 1.2489x; 1.0280x over previous
"""Multi-head attention kernel for Trainium2, 8-core tensor/data parallel.

Problem: x[2,2048,1024] -> qkv proj (w_qkv [1024,3072]) -> 16-head attention
         -> out proj (w_proj [1024,1024]) + b_proj.

Sharding: core c handles batch b=c//4 and heads 4*(c%4)..4*(c%4)+4.
Each core computes a partial output Y^T = w_proj_rows^T @ OH (its 4 heads'
contribution, transposed); the host sums the 4 partials per batch,
transposes, and adds the bias.

Schedule: the kernel is ACT-engine bound (softmax exp is ~147us of scalar
engine time per core and exp only runs there), so everything is organized
as a single slot stream paced by the exp chain.  Each slot emits one
S^T matmul pair (row-tiled K=64 halves that run concurrently on the PE
array), the exp for that tile, and the PV matmul pair for the slot LAG
positions earlier; Q/K/V projections and the output projection are woven
into the remaining PE capacity between slots using a ns-budget model so
the PE queue never runs ahead of the ACT queue (which would stall exp).
All data is bf16 (inputs, weights, activations, output partials); PSUM
accumulation stays fp32.  Softmax skips max-subtraction (scores are
~N(0,1) after the 1/sqrt(D) scale) and folds the row-sum into the PV
matmul via an appended ones-column on V; denominators use the fast
approximate reciprocal (~18 bits, plenty for this tolerance).
"""

from contextlib import ExitStack

import numpy as np

import concourse.bass as bass
import concourse.mybir as mybir
from concourse import bacc, tile

B, N, C, H = 2, 2048, 1024, 16
D = C // H            # 64 head dim
SCALE = float(D) ** -0.5
HPC = 4               # heads per core
HD = HPC * D          # 256 head-dim columns per core
NCORES = 8

F32 = mybir.dt.float32
F32R = mybir.dt.float32r
BF16 = mybir.dt.bfloat16

QT = N // 128         # 16 query/key 128-tiles
CT = C // 128         # 8 channel 128-tiles
QB = N // 512         # 4 query 512-blocks
HDT = HD // 128       # 2 head-pair tiles (2 heads of 64 each)

NIT = HDT * QB        # 8 attention iterations (ht-major)
LAG = 8               # PV trails its exp by this many slots

# empirical effective PE costs (ns), from HW trace at nominal clock
SPAIR_NS = 390.0      # row-tiled S matmul pair (2x [64]x512)
PV_NS = 225.0         # one PV matmul, ap=512
PROJ_MM_NS = 240.0    # projection matmul, ap=512
RB_NS = 250.0         # ones-broadcast matmul
EXP_NS = 1147.0       # ACT ns per [128,2,512] exp
AHEAD_NS = 2000.0     # how far PE emission may run ahead of ACT


def _build():
    nc = bacc.Bacc(None)
    xT = nc.declare_dram_parameter("xT", [C, N], BF16, isOutput=False)
    wkq = nc.declare_dram_parameter("wkq", [C, 2 * HD], BF16,
                                    isOutput=False)
    wv = nc.declare_dram_parameter("wv", [C, HD], BF16, isOutput=False)
    wp = nc.declare_dram_parameter("wp", [HD, C], BF16, isOutput=False)
    yT = nc.declare_dram_parameter("yT", [C, N], BF16, isOutput=True)

    with tile.TileContext(nc) as tc, ExitStack() as ctx:
        const_pool = ctx.enter_context(tc.tile_pool(name="const", bufs=1))
        w_pool = ctx.enter_context(tc.tile_pool(name="w", bufs=1))
        x_pool = ctx.enter_context(tc.tile_pool(name="x", bufs=1))
        qk_pool = ctx.enter_context(tc.tile_pool(name="qk", bufs=1))
        vo_pool = ctx.enter_context(tc.tile_pool(name="vo", bufs=1))
        oht_pool = ctx.enter_context(tc.tile_pool(name="oht", bufs=1))
        pt_pool = ctx.enter_context(tc.tile_pool(name="pt", bufs=1))
        small_pool = ctx.enter_context(tc.tile_pool(name="small", bufs=2))
        out_pool = ctx.enter_context(tc.tile_pool(name="out", bufs=2))
        st_pool = ctx.enter_context(
            tc.tile_pool(name="ps_st", bufs=2, space="PSUM"))
        ot_pool = ctx.enter_context(
            tc.tile_pool(name="ps_ot", bufs=1, space="PSUM"))
        proj_pool = ctx.enter_context(
            tc.tile_pool(name="ps_proj", bufs=2, space="PSUM"))

        ones_f = const_pool.tile([128, 64], F32)
        nc.vector.memset(ones_f, 1.0)
        ones_b = const_pool.tile([128, 64], BF16)
        nc.vector.memset(ones_b, 1.0)

        # ---- input DMAs, ordered by first use ----
        xq0_t = [x_pool.tile([128, CT // 2, 512], BF16, name=f"xq0{h}",
                             tag=f"xq0{h}") for h in range(2)]
        xq_t = [None] + [x_pool.tile([128, CT, 512], BF16, name=f"xq{qb}",
                                     tag=f"xq{qb}") for qb in range(1, QB)]
        wtiles = {}

        def xq(qb, ct):
            if qb == 0:
                return xq0_t[ct // (CT // 2)][:, ct % (CT // 2), :]
            return xq_t[qb][:, ct, :]

        # split input DMAs across the two hw DMA-gen engines (sync + scalar)
        # so transfers run on two queues in parallel; ACT is idle at startup
        def dma_w(name, dram, eng, width=HD):
            t = w_pool.tile([128, CT, width], BF16, name=name, tag=name)
            eng.dma_start(
                out=t, in_=dram[:, :].rearrange("(ct p) h -> p ct h", p=128))
            wtiles[name] = t

        def dma_x(qb, eng):
            qs = slice(qb * 512, (qb + 1) * 512)
            eng.dma_start(
                out=xq_t[qb],
                in_=xT[:, qs].rearrange("(ct p) n -> p ct n", p=128))

        # qb0 halves first on sync; wk then wq halves first on scalar
        for h in range(2):
            cs = slice(h * 512, h * 512 + 512)
            nc.sync.dma_start(
                out=xq0_t[h],
                in_=xT[cs, 0:512].rearrange("(ct p) n -> p ct n", p=128))
        wkq_tile = w_pool.tile([128, CT, 2 * HD], BF16, name="wkq",
                               tag="wkq")
        wtiles["wkq"] = wkq_tile
        for h in range(2):
            ws = slice(h * HD, (h + 1) * HD)
            nc.scalar.dma_start(
                out=wkq_tile[:, :, ws],
                in_=wkq[:, ws].rearrange("(ct p) h -> p ct h", p=128))
        dma_x(1, nc.sync)
        dma_w("wv", wv, nc.scalar)
        dma_x(2, nc.sync)
        dma_x(3, nc.scalar)
        wp_full = w_pool.tile([128, HDT, C], BF16, name="wp", tag="wp")
        nc.sync.dma_start(
            out=wp_full, in_=wp[:, :].rearrange("(ht p) c -> p ht c", p=128))

        # preload the exp table set while DMAs land (after the scalar-queue
        # DMA issues so it doesn't delay them)
        warm = const_pool.tile([128, 64], BF16)
        nc.scalar.activation(warm, ones_f, mybir.ActivationFunctionType.Exp,
                             scale=0.0)

        # ---- persistent activations (all bf16) ----
        qT_t = [qk_pool.tile([128, N], BF16, name=f"qT{i}", tag=f"qT{i}")
                for i in range(HDT)]
        kT_t = [qk_pool.tile([128, N], BF16, name=f"kT{i}", tag=f"kT{i}")
                for i in range(HDT)]
        vo_t = [vo_pool.tile([128, HPC * (D + 1)], BF16, name=f"vo{i}",
                             tag=f"vo{i}") for i in range(QT)]
        oht_t = [oht_pool.tile([128, N], BF16, name=f"oht{i}", tag=f"oht{i}")
                 for i in range(HDT)]
        pt_t = [pt_pool.tile([128, 2, 512], BF16, name=f"pt{i}",
                             tag=f"pt{i}") for i in range(QT)]
        yout = [None]  # current output staging tile

        for t in vo_t:
            ones_col = t.rearrange("p (h e) -> p h e", h=HPC)[:, :, D:D + 1]
            nc.gpsimd.tensor_copy(
                ones_col, ones_b[:, 0:HPC].rearrange("p (h o) -> p h o", o=1))

        # ---- work quanta ----
        def q_projqk(ht, qb, dst_t, woff):
            def go():
                w_full = wtiles["wkq"]
                cs = slice(qb * 512, (qb + 1) * 512)
                ps = proj_pool.tile([128, 512], F32, name="proj", tag="proj")
                for ct in range(CT):
                    nc.tensor.matmul(
                        ps,
                        w_full[:, ct, woff + ht * 128:woff + (ht + 1) * 128],
                        xq(qb, ct),
                        start=(ct == 0), stop=(ct == CT - 1))
                nc.vector.tensor_copy(dst_t[ht][:, cs], ps)
            return go, CT * PROJ_MM_NS

        def q_projv(kt):
            def go():
                qbk, off = divmod(kt * 128, 512)
                ks = slice(off, off + 128)
                ps = proj_pool.tile([128, 512], F32, name="proj", tag="proj")
                for ct in range(CT):
                    nc.tensor.matmul(ps[:, 0:HD], xq(qbk, ct)[:, ks],
                                     wtiles["wv"][:, ct, :],
                                     start=(ct == 0), stop=(ct == CT - 1))
                vo_view = vo_t[kt].rearrange("p (h e) -> p h e", h=HPC)
                ps_view = ps[:, 0:HD].rearrange("p (h d) -> p h d", h=HPC)
                nc.vector.tensor_copy(vo_view[:, :, 0:D], ps_view)
            return go, CT * PROJ_MM_NS

        def q_projout(qb, ct):
            def go():
                qs = slice(qb * 512, (qb + 1) * 512)
                cs = slice(ct * 128, (ct + 1) * 128)
                if ct == 0:
                    yout[0] = out_pool.tile([128, CT, 512], BF16, name="yo",
                                            tag="yo")
                ps = proj_pool.tile([128, 512], F32, name="proj", tag="proj")
                for ht in range(HDT):
                    nc.tensor.matmul(
                        ps, wp_full[:, ht, cs], oht_t[ht][:, qs],
                        start=(ht == 0), stop=(ht == HDT - 1))
                nc.vector.tensor_copy(yout[0][:, ct, :], ps)
                if ct == CT // 2 - 1 or ct == CT - 1:
                    h = ct // (CT // 2)
                    cs2 = slice(h * 512, h * 512 + 512)
                    nc.sync.dma_start(
                        out=yT[cs2, qs].rearrange("(ct p) n -> p ct n",
                                                  p=128),
                        in_=yout[0][:, h * (CT // 2):(h + 1) * (CT // 2), :])
            return go, HDT * PROJ_MM_NS

        # ---- slot-stream emission ----
        state = {"pe": 0.0, "act": 0.0}
        fifo = []          # [(deadline, go, cost), ...] kept sorted
        done_ids = set()

        def push(deadline, qid, quantum):
            go, cost = quantum
            fifo.append([deadline, qid, go, cost])
            fifo.sort(key=lambda e: e[0])

        def run_item(item):
            _, qid, go, cost = item
            go()
            state["pe"] += cost
            done_ids.add(qid)

        def force_until(g):
            while fifo and fifo[0][0] <= g:
                run_item(fifo.pop(0))

        def budget_drain():
            while fifo and state["pe"] + fifo[0][3] <= state["act"] + AHEAD_NS:
                run_item(fifo.pop(0))

        # prologue projections for (ht0, qb0): kT cols 0-255 first (covers
        # kt0/kt1), then full qT, then the kT remainder -- the first S pair
        # only needs the narrow kT chunk plus qT
        def projqk_chunk(dst_t, woff, lo, hi):
            w_full = wtiles["wkq"]
            ps = proj_pool.tile([128, 512], F32, name="proj", tag="proj")
            for ct in range(CT):
                nc.tensor.matmul(
                    ps[:, 0:hi - lo],
                    w_full[:, ct, woff:woff + 128],
                    xq(0, ct)[:, lo:hi],
                    start=(ct == 0), stop=(ct == CT - 1))
            nc.vector.tensor_copy(dst_t[0][:, lo:hi], ps[:, 0:hi - lo])
            state["pe"] += CT * PROJ_MM_NS * (hi - lo) / 512.0

        projqk_chunk(kT_t, 0, 0, 256)
        projqk_chunk(qT_t, HD, 0, 512)
        projqk_chunk(kT_t, 0, 256, 512)

        # weave queue: deadlines in global slot units
        for qbk in range(1, QB):
            push(4 * qbk, ("kT", 0, qbk), q_projqk(0, qbk, kT_t, 0))
        for kt in range(QT):
            push(kt + LAG, ("v", kt), q_projv(kt))
        for qb in range(1, QB):
            push(16 * qb, ("qT", 0, qb), q_projqk(0, qb, qT_t, HD))
        for qbk in range(QB):
            push(64 + 4 * qbk - 8, ("kT", 1, qbk), q_projqk(1, qbk, kT_t, 0))
        for qb in range(QB):
            push(64 + 16 * qb, ("qT", 1, qb), q_projqk(1, qb, qT_t, HD))

        iters = [(ht, qb) for ht in range(HDT) for qb in range(QB)]
        ots_by_it = {}
        norm_pending = []
        normb_pending = []
        pv_sched = {}
        for it in range(NIT):
            lag = 6 if it == NIT - 1 else LAG
            for kt in range(QT):
                pv_sched.setdefault(it * QT + kt + lag, []).append((it, kt))

        def emit_s_exp(it, kt):
            ht, qb = iters[it]
            qs = slice(qb * 512, (qb + 1) * 512)
            st = st_pool.tile([128, 2, 512], F32, name="st", tag="st",
                              bufs=2)
            for hp in range(2):
                prow = slice(hp * 64, hp * 64 + 64)
                nc.tensor.matmul(
                    st[:, hp, :],
                    kT_t[ht][prow, kt * 128:(kt + 1) * 128],
                    qT_t[ht][prow, qs])
            nc.scalar.activation(
                pt_t[kt], st, mybir.ActivationFunctionType.Exp, scale=SCALE)
            state["pe"] += SPAIR_NS
            state["act"] += EXP_NS

        def emit_pv(it, kt):
            ht, qb = iters[it]
            if kt == 0:
                ots_by_it[it] = [
                    ot_pool.tile([65, 512], F32, name=f"ot{hp}",
                                 tag=f"ot{hp}", bufs=1)
                    for hp in range(2)]
            ots = ots_by_it[it]
            for hp in range(2):
                h = 2 * ht + hp
                nc.tensor.matmul(
                    ots[hp],
                    vo_t[kt][:, h * (D + 1):(h + 1) * (D + 1)],
                    pt_t[kt][:, hp, :],
                    start=(kt == 0), stop=(kt == QT - 1))
            state["pe"] += 2 * PV_NS
            if kt == QT - 1:
                norm_pending.append(it)

        def emit_norm_a(it):
            # free the PSUM accumulators ASAP: stage O + rowsum to SBUF.
            # rowsum goes to its own partition-0 tile: reciprocal_approx_fast
            # breaks on nonzero base partitions as well as on PSUM reads.
            stgs = []
            ots = ots_by_it.pop(it)
            for hp in range(2):
                stg = small_pool.tile([64, 512], F32, name=f"stg{hp}",
                                      tag=f"stg{hp}")
                nc.vector.tensor_copy(stg, ots[hp][0:64, :])
                sdb = small_pool.tile([1, 512], F32, name=f"sd{hp}",
                                      tag=f"sd{hp}")
                nc.vector.tensor_copy(sdb, ots[hp][64:65, :])
                stgs.append((stg, sdb))
            return stgs

        def emit_norm_b(it, stgs):
            ht, qb = iters[it]
            qs = slice(qb * 512, (qb + 1) * 512)
            rbs = []
            for hp in range(2):
                stg, sdb = stgs[hp]
                r32 = small_pool.tile([1, 512], F32, name=f"r32{hp}",
                                      tag=f"r32{hp}")
                # approx_fast's bit-trick seed reads garbage through the
                # PSUM port -- it must read SBUF at partition 0 (HW-verified)
                nc.vector.reciprocal_approx_fast(r32, sdb)
                # replicate 1/rowsum across partitions on the idle GPSIMD
                # engine (HW-verified bit-faithful); keeps norm off the PE
                rb = small_pool.tile([64, 512], F32, name=f"rb{hp}",
                                     tag=f"rb{hp}")
                nc.gpsimd.partition_broadcast(rb, r32)
                rbs.append(rb)
            for hp in range(2):
                prow = slice(hp * 64, hp * 64 + 64)
                dst = oht_t[ht][prow, qs]
                with nc.allow_low_precision(reason="bf16 attention out"):
                    nc.vector.tensor_mul(dst, stgs[hp][0][0:64, :], rbs[hp])
            if ht == HDT - 1:
                base = (4 + qb) * 16 + 32
                for ct in range(CT):
                    dl = base + 2 * ct if qb < QB - 1 else 10 ** 6
                    push(dl, ("out", qb, ct), q_projout(qb, ct))

        total_slots = NIT * QT
        for g in range(total_slots + LAG):
            force_until(g)
            if g < total_slots:
                it, kt = divmod(g, QT)
                emit_s_exp(it, kt)
            if norm_pending:
                itn = norm_pending.pop(0)
                normb_pending.append((g + 2, itn, emit_norm_a(itn)))
            if normb_pending and normb_pending[0][0] <= g:
                _, itn, stgs = normb_pending.pop(0)
                emit_norm_b(itn, stgs)
            for gpv in pv_sched.pop(g, ()):
                emit_pv(*gpv)
            budget_drain()
        if norm_pending:
            itn = norm_pending.pop(0)
            emit_norm_b(itn, emit_norm_a(itn))
        while normb_pending:
            _, itn, stgs = normb_pending.pop(0)
            emit_norm_b(itn, stgs)
        while fifo:
            run_item(fifo.pop(0))

    nc.finalize()
    return nc


_NC_CACHE = None
TRACE = False
LAST_RESULTS = None


def _get_nc():
    global _NC_CACHE
    if _NC_CACHE is None:
        _NC_CACHE = _build()
    return _NC_CACHE


def kernel(x, w_qkv, w_proj, b_proj):
    global LAST_RESULTS
    import ml_dtypes
    from concourse.bass_utils import run_bass_kernel_spmd

    BF = ml_dtypes.bfloat16
    x = np.asarray(x, dtype=np.float32)
    w_qkv = np.asarray(w_qkv, dtype=np.float32)
    w_proj = np.asarray(w_proj, dtype=np.float32)
    b_proj = np.asarray(b_proj, dtype=np.float32)

    nc = _get_nc()
    xT_b = [np.ascontiguousarray(x[b].T.astype(BF)) for b in range(B)]
    in_maps = []
    for c in range(NCORES):
        b, g = divmod(c, NCORES // B)
        hs = slice(g * HD, (g + 1) * HD)
        wk_g = w_qkv[:, 1 * C:2 * C][:, hs]
        wq_g = w_qkv[:, 0 * C:1 * C][:, hs]
        in_maps.append({
            "xT": xT_b[b],
            "wkq": np.ascontiguousarray(
                np.concatenate([wk_g, wq_g], axis=1).astype(BF)),
            "wv": np.ascontiguousarray(w_qkv[:, 2 * C:3 * C][:, hs].astype(BF)),
            "wp": np.ascontiguousarray(w_proj[g * HD:(g + 1) * HD, :].astype(BF)),
        })
    res = run_bass_kernel_spmd(nc, in_maps, list(range(NCORES)), trace=TRACE)
    LAST_RESULTS = res
    out = np.empty((B, N, C), dtype=np.float32)
    ncb = NCORES // B
    for b in range(B):
        acc = res.results[b * ncb]["yT"].astype(np.float32)
        for g in range(1, ncb):
            acc += res.results[b * ncb + g]["yT"].astype(np.float32)
        out[b] = acc.T + b_proj
    return out
